# revision 1
# baseline (speedup 1.0000x reference)
"""Multi-head causal attention (B=4, S=2048, D=1024, H=16) for 8 Trainium2 cores.

Sharding: core c = (batch b = c//2, head-group g = c%2). Each core computes,
for its batch and its 8 heads: QKV projections, causal softmax attention, and
a partial output projection (its heads' rows of Wo). Host sums the two
head-group partials per batch and adds the output bias.

Device layout notes:
 - All matmuls run as float32r (full PE rate, ~1.5e-4 rel precision).
 - Scores are computed transposed, ST[k, q] = K Q^T, so softmax's reduction
   over keys lands on the partition axis where it is fused into the PV matmul
   via an extra ones-column of V (denominator accumulates in PSUM row 64).
 - Causal mask applied by accumulating -BIG * (k + 128j - q)^+ into the score
   PSUM with one extra matmul per diagonal chunk (A strictly-lower-triangular
   0/1 matrix times a shifted step matrix of -BIG).
 - softmax normalization: denominators DMA'd into a per-qt staging tile,
   reciprocal on DVE (approx, ~51 ULP), broadcast across partitions on
   GPSIMD, applied with DVE multiplies.
"""

import sys

if "/opt/trn_rl_repo" not in sys.path:
    sys.path.insert(0, "/opt/trn_rl_repo")

import numpy as np

B, S, D = 4, 2048, 1024
H, DH = 16, 64
NCORES = 8
GH = H // 2            # heads per core
GW = GH * DH           # head-group width (512)
NP = GW // 128         # head pairs per core (4)
SM_SCALE = float(1.0 / np.sqrt(np.float32(D)))
BIG = 1.0e30


def build_mha_kernel(S_, D_, debug=False, debug_taps=False):
    import concourse.bass as bass  # noqa: F401
    import concourse.mybir as mybir
    import concourse.tile as tile
    from concourse import bacc

    f32 = mybir.dt.float32
    f32r = mybir.dt.float32r

    KT = D_ // 128          # input-dim tiles
    NQT = S_ // 512         # q tiles
    NST = S_ // 512         # s tiles for streaming XT in phase 1

    nc = bacc.Bacc("TRN2", target_bir_lowering=False, debug=debug)

    XT_d = nc.dram_tensor("XT", [D_, S_], f32, kind="ExternalInput")
    WQ_d = nc.dram_tensor("WQ", [D_, GW], f32, kind="ExternalInput")
    WK_d = nc.dram_tensor("WK", [D_, GW], f32, kind="ExternalInput")
    WV_d = nc.dram_tensor("WV", [D_, GW], f32, kind="ExternalInput")
    WO_d = nc.dram_tensor("WO", [GW, D_], f32, kind="ExternalInput")
    AT_d = nc.dram_tensor("AT", [128, 128], f32, kind="ExternalInput")
    A2_d = nc.dram_tensor("A2", [128, 128], f32, kind="ExternalInput")
    BB_d = nc.dram_tensor("BB", [128, 640], f32, kind="ExternalInput")
    ON_d = nc.dram_tensor("ON", [128, S_ // 128 * GH], f32, kind="ExternalInput")
    Y_d = nc.dram_tensor("Y", [S_, D_], f32, kind="ExternalOutput")
    if debug_taps:
        QTD = nc.dram_tensor("QTD", [128, NP, S_], f32, kind="ExternalOutput")
        VD = nc.dram_tensor("VD", [128, S_ // 128, GH * 65], f32,
                            kind="ExternalOutput")
        PTD = nc.dram_tensor("PTD", [128, 2, 512], f32, kind="ExternalOutput")
        CTXD = nc.dram_tensor("CTXD", [128, 1024], f32, kind="ExternalOutput")
        DEND = nc.dram_tensor("DEND", [1, 1024], f32, kind="ExternalOutput")
        RECD = nc.dram_tensor("RECD", [1, 1024], f32, kind="ExternalOutput")
        BCAD = nc.dram_tensor("BCAD", [64, 512], f32, kind="ExternalOutput")
        CTXND = nc.dram_tensor("CTXND", [128, NP, 512], f32,
                               kind="ExternalOutput")

    Exp = mybir.ActivationFunctionType.Exp

    with tile.TileContext(nc) as tc:
        with tc.tile_pool(name="const", bufs=1) as const_pool, \
             tc.tile_pool(name="big", bufs=1) as big_pool:

            # ---- persistent activations ----
            QT_t = big_pool.tile([128, NP, S_], f32r)     # Q^T  [dout, s]
            KT_t = big_pool.tile([128, NP, S_], f32r)     # K^T  [dout, s]
            NKC = S_ // 128
            V_t = big_pool.tile([128, NKC, GH * 65], f32r)  # V + ones col per head

            # ---- phase 1: QKV projections (stream XT by s-tiles of 512) ----
            with tc.tile_pool(name="xw", bufs=1) as xw_pool, \
                 tc.tile_pool(name="ps1", bufs=8, space="PSUM") as ps1_pool:
                WQ_t = xw_pool.tile([128, KT, GW], f32r, tag="wq")
                WK_t = xw_pool.tile([128, KT, GW], f32r, tag="wk")
                WV_t = xw_pool.tile([128, KT, GW], f32r, tag="wv")
                XT_r = XT_d.rearrange("(kt p) s -> p kt s", p=128).bitcast(f32r)
                WQ_r = WQ_d.rearrange("(kt p) n -> p kt n", p=128).bitcast(f32r)
                # first s-tile + per-chunk WQ DMAs issued first so the first
                # matmuls can start as soon as possible
                xt0 = xw_pool.tile([128, KT, 512], f32r, tag="xt", bufs=2)
                WK_r = WK_d.rearrange("(kt p) n -> p kt n", p=128).bitcast(f32r)
                WV_r = WV_d.rearrange("(kt p) n -> p kt n", p=128).bitcast(f32r)
                for kt in range(KT):
                    nc.sync.dma_start(xt0[:, kt], XT_r[:, kt, 0:512])
                    nc.sync.dma_start(WQ_t[:, kt], WQ_r[:, kt])
                    nc.sync.dma_start(WK_t[:, kt], WK_r[:, kt])
                    nc.sync.dma_start(WV_t[:, kt], WV_r[:, kt])

                for st in range(NST):
                    if st == 0:
                        xt = xt0
                    else:
                        xt = xw_pool.tile([128, KT, 512], f32r, tag="xt", bufs=2)
                        for kt in range(KT):
                            nc.sync.dma_start(
                                xt[:, kt], XT_r[:, kt, st * 512:(st + 1) * 512])
                    for c in range(NP):
                        for which, wt, outt in (("q", WQ_t, QT_t), ("k", WK_t, KT_t)):
                            psqk = ps1_pool.tile([128, 512], f32, tag="ps1")
                            for kt in range(KT):
                                nc.tensor.matmul(
                                    psqk[:, :],
                                    lhsT=wt[:, kt, c * 128:(c + 1) * 128],
                                    rhs=xt[:, kt, :],
                                    start=(kt == 0), stop=(kt == KT - 1))
                            nc.vector.tensor_copy(
                                out=outt[:, c, st * 512:(st + 1) * 512],
                                in_=psqk[:, :])
                    for sc in range(4):
                        scc = st * 4 + sc
                        psv = ps1_pool.tile([128, 512], f32, tag="ps1")
                        for kt in range(KT):
                            nc.tensor.matmul(
                                psv[:, :],
                                lhsT=xt[:, kt, sc * 128:(sc + 1) * 128],
                                rhs=WV_t[:, kt, :],
                                start=(kt == 0), stop=(kt == KT - 1))
                        nc.vector.tensor_copy(
                            out=V_t[:, scc].rearrange(
                                "p (h e) -> p h e", e=65)[:, :, 0:64],
                            in_=psv[:, :].rearrange("p (h d) -> p h d", d=64))

                # masks / WO / ones loads (needed from attention onward)
                atri = const_pool.tile([128, 128], f32r)  # A[r,k] = 1 iff r < k
                nc.sync.dma_start(atri, AT_d[:].bitcast(f32r))
                atri2 = const_pool.tile([128, 128], f32r)  # A2[r,k] = 1 iff r <= k
                nc.sync.dma_start(atri2, A2_d[:].bitcast(f32r))
                # B[r,c] = -BIG iff r >= c - 128 (extended step matrix; the
                # usual slice starts at col 128, the clamped-trim slice at 0)
                bbig = const_pool.tile([128, 640], f32r)
                nc.sync.dma_start(bbig, BB_d[:].bitcast(f32r))
                WO_t = const_pool.tile([128, NP, D_], f32r)
                nc.sync.dma_start(
                    WO_t, WO_d.rearrange("(c p) n -> p c n", p=128).bitcast(f32r))
                nc.sync.dma_start(
                    V_t.rearrange("p kc (h e) -> p (kc h) e", e=65)[:, :, 64:65],
                    ON_d[:, :, None].bitcast(f32r))

            if debug_taps:
                nc.sync.dma_start(QTD[:], QT_t.bitcast(f32))
                nc.sync.dma_start(VD[:], V_t.bitcast(f32))

            # ---- phase 2: attention + output projection, per q-tile ----
            with tc.tile_pool(name="att", bufs=6) as att_pool, \
                 tc.tile_pool(name="nrm", bufs=2) as nrm_pool, \
                 tc.tile_pool(name="ps", bufs=4, space="PSUM") as ps_pool:
                def emit_oproj(qt, ctxn, sss=(0, 1, 2, 3)):
                    NOUT = max(1, D_ // 512)
                    OW = min(512, D_)
                    for ss in sss:
                        yp = ps_pool.tile([128, 1024], f32, tag="ps")
                        for c in range(NP):
                            for n in range(NOUT):
                                nc.tensor.matmul(
                                    yp[:, n * OW:(n + 1) * OW],
                                    lhsT=ctxn[c][:, ss * 128:(ss + 1) * 128],
                                    rhs=WO_t[:, c, n * OW:(n + 1) * OW],
                                    start=(c == 0), stop=(c == NP - 1))
                        ys = nrm_pool.tile([128, NOUT * OW], f32, tag="ys")
                        if ss % 2 == 0:
                            nc.scalar.copy(out=ys, in_=yp[:, 0:NOUT * OW])
                        else:
                            nc.vector.tensor_copy(out=ys, in_=yp[:, 0:NOUT * OW])
                        nc.sync.dma_start(
                            Y_d[qt * 512 + ss * 128: qt * 512 + (ss + 1) * 128, :],
                            ys)

                prev = None
                qt_order = [2, 3, 1, 0] if NQT == 4 else list(range(NQT))
                for qt in qt_order:
                    nkc = 4 * qt + 4
                    ctxn = [nrm_pool.tile([128, 512], f32r, tag=f"ctxn{c}",
                                          name=f"ctxn{c}_{qt}")
                            for c in range(NP)]
                    for c in range(NP):
                        ctx = ps_pool.tile([128, 1024], f32, tag="ps")
                        for kc in range(nkc):
                            jp = kc - 4 * qt
                            # f32r matmuls run at 1/4 rate below N=256, so
                            # never trim past 256; the mask matmul zeroes the
                            # extra (fully-masked) columns exactly via exp.
                            trim = min(128 * jp, 256) if jp >= 0 else 0
                            shift = (128 * jp - trim) if jp >= 0 else 0
                            qs = qt * 512
                            stp = ps_pool.tile([128, 1024], f32, tag="ps")
                            st2 = stp.rearrange("p (i n) -> p i n", n=512)
                            for i, lo in ((0, 0), (1, 64)):
                                nc.tensor.matmul(
                                    st2[:, i, trim:512],
                                    lhsT=KT_t[lo:lo + 64, c,
                                              kc * 128:(kc + 1) * 128],
                                    rhs=QT_t[lo:lo + 64, c, qs + trim:qs + 512],
                                    start=True, stop=(jp < 0),
                                    skip_group_check=True)
                            if jp >= 0:
                                # shift>0 (clamped trim): use A2 = (r <= k)
                                # with B shifted one col so the k=0 row of
                                # the fully-masked band still masks.
                                mA = atri2 if shift else atri
                                blo = 1 if shift else 128
                                for i in (0, 1):
                                    nc.tensor.matmul(
                                        st2[:, i, trim:512],
                                        lhsT=mA,
                                        rhs=bbig[:, blo:blo + 512 - trim],
                                        start=False, stop=True,
                                        skip_group_check=True)
                            pt = att_pool.tile([128, 2, 512], f32r, tag="pt")
                            nc.scalar.activation(
                                pt[:, :, trim:512], st2[:, :, trim:512],
                                Exp, scale=SM_SCALE)
                            if debug_taps and qt == 0 and c == 0 and kc == 0:
                                nc.sync.dma_start(PTD[:], pt.bitcast(f32))
                            for i in (0, 1):
                                h = 2 * c + i
                                nc.tensor.matmul(
                                    ctx[0:65, i * 512 + trim:i * 512 + 512],
                                    lhsT=V_t[:, kc, h * 65:(h + 1) * 65],
                                    rhs=pt[:, i, trim:512],
                                    start=(kc == 0), stop=(kc == nkc - 1),
                                    skip_group_check=True)
                        if debug_taps and qt == 0 and c == 0:
                            ctxdump = nrm_pool.tile([128, 1024], f32,
                                                    tag="ctxdump")
                            nc.vector.tensor_copy(out=ctxdump, in_=ctx)
                            nc.sync.dma_start(CTXD[:], ctxdump)
                        # denominators live in psum row 64; stage to SBUF on
                        # the same partition, DMA-shift to partition 0 (the
                        # custom-DVE recip and gpsimd broadcast only handle
                        # base-partition-0 inputs on HW), then broadcast.
                        den = nrm_pool.tile([65, 1024], f32, tag="den")
                        nc.vector.tensor_copy(out=den[64:65, :],
                                              in_=ctx[64:65, 0:1024])
                        den0 = nrm_pool.tile([1, 1024], f32, tag="den0")
                        nc.sync.dma_start(den0, den[64:65, :])
                        recip = nrm_pool.tile([1, 1024], f32, tag="recip")
                        nc.vector.reciprocal_approx_fast(out=recip, in_=den0)
                        bca = nrm_pool.tile([64, 512], f32, tag="bca")
                        bcb = nrm_pool.tile([64, 512], f32, tag="bcb")
                        nc.gpsimd.partition_broadcast(bca, recip[0:1, 0:512])
                        nc.gpsimd.partition_broadcast(bcb, recip[0:1, 512:1024])
                        if debug_taps and qt == 0 and c == 0:
                            nc.sync.dma_start(DEND[:], den[64:65, :])
                            nc.sync.dma_start(RECD[:], recip)
                            nc.sync.dma_start(BCAD[:], bca)
                        nc.vector.tensor_mul(
                            ctxn[c][0:64, :], ctx[0:64, 0:512], bca)
                        tmpb = nrm_pool.tile([64, 512], f32r, tag="tmpb")
                        nc.vector.tensor_mul(
                            tmpb, ctx[0:64, 512:1024], bcb)
                        nc.sync.dma_start(ctxn[c][64:128, :], tmpb)
                        if c == 1 and prev is not None:
                            emit_oproj(*prev)
                            prev = None

                    if debug_taps and qt == 0:
                        for c in range(NP):
                            nc.sync.dma_start(CTXND[:, c, :],
                                              ctxn[c].bitcast(f32))
                    prev = (qt, ctxn)
                if prev is not None:
                    emit_oproj(*prev)

    nc.compile()
    return nc


_NC_CACHE = {}


def _get_nc():
    key = (S, D)
    if key not in _NC_CACHE:
        _NC_CACHE[key] = build_mha_kernel(S, D)
    return _NC_CACHE[key]


def make_consts(S_):
    r = np.arange(128)
    at = (r[:, None] < r[None, :]).astype(np.float32)          # A[r,k] = r < k
    at2 = (r[:, None] <= r[None, :]).astype(np.float32)        # A2[r,k] = r <= k
    bb = np.where(r[:, None] >= np.arange(640)[None, :] - 128,
                  np.float32(-BIG), np.float32(0.0)).astype(np.float32)
    on = np.ones((128, S_ // 128 * GH), dtype=np.float32)
    return at, at2, bb, on


def shard_inputs(X, Wq, Wk, Wv, Wo):
    """Build the 8 per-core input maps from full inputs."""
    X = np.asarray(X, dtype=np.float32)
    Wq = np.asarray(Wq, dtype=np.float32)
    Wk = np.asarray(Wk, dtype=np.float32)
    Wv = np.asarray(Wv, dtype=np.float32)
    Wo = np.asarray(Wo, dtype=np.float32)
    at, at2, bb, on = make_consts(S)
    in_maps = []
    for c in range(NCORES):
        b, g = c // 2, c % 2
        in_maps.append({
            "XT": np.ascontiguousarray(X[b].T),
            "WQ": np.ascontiguousarray(Wq[:, g * GW:(g + 1) * GW]),
            "WK": np.ascontiguousarray(Wk[:, g * GW:(g + 1) * GW]),
            "WV": np.ascontiguousarray(Wv[:, g * GW:(g + 1) * GW]),
            "WO": np.ascontiguousarray(Wo[g * GW:(g + 1) * GW, :]),
            "AT": at, "A2": at2, "BB": bb, "ON": on,
        })
    return in_maps


def kernel(X, Wq, Wk, Wv, Wo, bo):
    from concourse.bass_utils import run_bass_kernel_spmd

    nc = _get_nc()
    in_maps = shard_inputs(X, Wq, Wk, Wv, Wo)
    res = run_bass_kernel_spmd(nc, in_maps, core_ids=list(range(NCORES)))
    bo = np.asarray(bo, dtype=np.float32)
    Y = np.empty((B, S, D), dtype=np.float32)
    for b in range(B):
        Y[b] = res.results[2 * b]["Y"] + res.results[2 * b + 1]["Y"] + bo
    return Y



# revision 11
# speedup vs baseline: 1.0205x; 1.0205x over previous
"""Multi-head causal attention (B=4, S=2048, D=1024, H=16) for 8 Trainium2 cores.

Sharding: core c = (batch b = c//2, head-group g = c%2). Each core computes,
for its batch and its 8 heads: QKV projections, causal softmax attention, and
a partial output projection (its heads' rows of Wo). Host sums the two
head-group partials per batch and adds the output bias.

Device layout notes:
 - All matmuls run as float32r (full PE rate, ~1.5e-4 rel precision).
 - Scores are computed transposed, ST[k, q] = K Q^T, so softmax's reduction
   over keys lands on the partition axis where it is fused into the PV matmul
   via an extra ones-column of V (denominator accumulates in PSUM row 64).
 - Causal mask applied by accumulating -BIG * (k + 128j - q)^+ into the score
   PSUM with one extra matmul per diagonal chunk (A strictly-lower-triangular
   0/1 matrix times a shifted step matrix of -BIG).
 - softmax normalization: denominators DMA'd into a per-qt staging tile,
   reciprocal on DVE (approx, ~51 ULP), broadcast across partitions on
   GPSIMD, applied with DVE multiplies.
"""

import sys

if "/opt/trn_rl_repo" not in sys.path:
    sys.path.insert(0, "/opt/trn_rl_repo")

import numpy as np

B, S, D = 4, 2048, 1024
H, DH = 16, 64
NCORES = 8
GH = H // 2            # heads per core
GW = GH * DH           # head-group width (512)
NP = GW // 128         # head pairs per core (4)
SM_SCALE = float(1.0 / np.sqrt(np.float32(D)))
BIG = 1.0e30
MBIG = 240.0           # fp8e4m3 (ieee) max finite; mask A=4 -> -960*cnt


def build_mha_kernel(S_, D_, debug=False, debug_taps=False):
    import concourse.bass as bass  # noqa: F401
    import concourse.mybir as mybir
    import concourse.tile as tile
    from concourse import bacc

    f32 = mybir.dt.float32
    f32r = mybir.dt.float32r
    f8 = mybir.dt.float8e4
    DR = mybir.MatmulPerfMode.DoubleRow

    KT = D_ // 128          # input-dim tiles
    NQT = S_ // 512         # q tiles
    NST = S_ // 512         # s tiles for streaming XT in phase 1

    nc = bacc.Bacc("TRN2", target_bir_lowering=False, debug=debug)

    XT_d = nc.dram_tensor("XT", [D_, S_], f32, kind="ExternalInput")
    WQ_d = nc.dram_tensor("WQ", [D_, GW], f32, kind="ExternalInput")
    WK_d = nc.dram_tensor("WK", [D_, GW], f32, kind="ExternalInput")
    WV_d = nc.dram_tensor("WV", [D_, GW], f32, kind="ExternalInput")
    WO_d = nc.dram_tensor("WO", [GW, D_], f32, kind="ExternalInput")
    AT_d = nc.dram_tensor("AT", [64, 2, 128], f8, kind="ExternalInput")
    A2_d = nc.dram_tensor("A2", [64, 2, 128], f8, kind="ExternalInput")
    BB_d = nc.dram_tensor("BB", [64, 2, 640], f8, kind="ExternalInput")
    ON_d = nc.dram_tensor("ON", [128, S_ // 128 * GH], f32, kind="ExternalInput")
    Y_d = nc.dram_tensor("Y", [S_, D_], f32, kind="ExternalOutput")
    if debug_taps:
        QTD = nc.dram_tensor("QTD", [128, NP, S_], f32, kind="ExternalOutput")
        VD = nc.dram_tensor("VD", [128, S_ // 128, GH * 65], f32,
                            kind="ExternalOutput")
        PTD = nc.dram_tensor("PTD", [128, 2, 512], f32, kind="ExternalOutput")
        CTXD = nc.dram_tensor("CTXD", [128, 1024], f32, kind="ExternalOutput")
        DEND = nc.dram_tensor("DEND", [1, 1024], f32, kind="ExternalOutput")
        RECD = nc.dram_tensor("RECD", [1, 1024], f32, kind="ExternalOutput")
        BCAD = nc.dram_tensor("BCAD", [64, 512], f32, kind="ExternalOutput")
        CTXND = nc.dram_tensor("CTXND", [128, NP, 512], f32,
                               kind="ExternalOutput")

    Exp = mybir.ActivationFunctionType.Exp

    with tile.TileContext(nc) as tc:
        with tc.tile_pool(name="const", bufs=1) as const_pool, \
             tc.tile_pool(name="big", bufs=1) as big_pool:

            # ---- persistent activations ----
            # Q^T/K^T live only as fp8e4m3 in DoubleRow-folded layout:
            # QDR[h][32c+pp, i, s] = Q^T[dh, s] of head pair c, head h, with
            # dh enumerated as (pp, i) pairs (any consistent order works --
            # the score contraction is invariant to dh relabeling).
            QDR = [big_pool.tile([128, 2, S_], f8, name=f"qdr{h}")
                   for h in (0, 1)]
            KDR = [big_pool.tile([128, 2, S_], f8, name=f"kdr{h}")
                   for h in (0, 1)]
            NKC = S_ // 128
            V_t = big_pool.tile([128, NKC, GH * 65], f32r)  # V + ones col per head

            # ---- phase 1: QKV projections (stream XT by s-tiles of 512) ----
            with tc.tile_pool(name="xw", bufs=1) as xw_pool, \
                 tc.tile_pool(name="ps1", bufs=8, space="PSUM") as ps1_pool:
                WQ_t = xw_pool.tile([128, KT, GW], f32r, tag="wq")
                WK_t = xw_pool.tile([128, KT, GW], f32r, tag="wk")
                WV_t = xw_pool.tile([128, KT, GW], f32r, tag="wv")
                # fp8 staging for Q^T/K^T (psum rows r -> (pp=r//2, i=r%2)
                # via the remap DMAs below)
                QF8 = xw_pool.tile([128, NP, S_], f8, tag="qf8")
                KF8 = xw_pool.tile([128, NP, S_], f8, tag="kf8")
                XT_r = XT_d.rearrange("(kt p) s -> p kt s", p=128).bitcast(f32r)
                WQ_r = WQ_d.rearrange("(kt p) n -> p kt n", p=128).bitcast(f32r)
                # first s-tile + per-chunk WQ DMAs issued first so the first
                # matmuls can start as soon as possible
                xt0 = xw_pool.tile([128, KT, 512], f32r, tag="xt", bufs=2)
                WK_r = WK_d.rearrange("(kt p) n -> p kt n", p=128).bitcast(f32r)
                WV_r = WV_d.rearrange("(kt p) n -> p kt n", p=128).bitcast(f32r)
                for kt in range(KT):
                    nc.sync.dma_start(xt0[:, kt], XT_r[:, kt, 0:512])
                    nc.sync.dma_start(WQ_t[:, kt], WQ_r[:, kt])
                    nc.sync.dma_start(WK_t[:, kt], WK_r[:, kt])
                    nc.sync.dma_start(WV_t[:, kt], WV_r[:, kt])

                for st in range(NST):
                    if st == 0:
                        xt = xt0
                    else:
                        xt = xw_pool.tile([128, KT, 512], f32r, tag="xt", bufs=2)
                        for kt in range(KT):
                            nc.sync.dma_start(
                                xt[:, kt], XT_r[:, kt, st * 512:(st + 1) * 512])
                    sl = slice(st * 512, (st + 1) * 512)
                    for c in range(NP):
                        for which, wt, outt, drt in (
                                ("q", WQ_t, QF8, QDR), ("k", WK_t, KF8, KDR)):
                            psqk = ps1_pool.tile([128, 512], f32, tag="ps1")
                            for kt in range(KT):
                                nc.tensor.matmul(
                                    psqk[:, :],
                                    lhsT=wt[:, kt, c * 128:(c + 1) * 128],
                                    rhs=xt[:, kt, :],
                                    start=(kt == 0), stop=(kt == KT - 1))
                            # cast f32 psum -> fp8 on the (phase-1-idle) Act
                            nc.scalar.copy(out=outt[:, c, sl], in_=psqk[:, :])
                            # partition fold 128 -> (64, 2): row 64h+2pp+i of
                            # the staging tile lands at DR[h][32c+pp, i, :]
                            for h in (0, 1):
                                nc.gpsimd.dma_start(
                                    drt[h][32 * c:32 * c + 32, :, sl],
                                    outt[64 * h:64 * h + 64, c, sl])
                    for sc in range(4):
                        scc = st * 4 + sc
                        psv = ps1_pool.tile([128, 512], f32, tag="ps1")
                        for kt in range(KT):
                            nc.tensor.matmul(
                                psv[:, :],
                                lhsT=xt[:, kt, sc * 128:(sc + 1) * 128],
                                rhs=WV_t[:, kt, :],
                                start=(kt == 0), stop=(kt == KT - 1))
                        nc.vector.tensor_copy(
                            out=V_t[:, scc].rearrange(
                                "p (h e) -> p h e", e=65)[:, :, 0:64],
                            in_=psv[:, :].rearrange("p (h d) -> p h d", d=64))

                # masks / WO / ones loads (needed from attention onward)
                # A[r,k] = 4 iff r < k, DR-folded as [64, 2, 128] (r = 64i+p)
                atri = const_pool.tile([64, 2, 128], f8)
                nc.sync.dma_start(atri, AT_d[:])
                atri2 = const_pool.tile([64, 2, 128], f8)  # A2: 4 iff r <= k
                nc.sync.dma_start(atri2, A2_d[:])
                # B[r,c] = -240 iff r >= c - 128, DR-folded [64, 2, 640];
                # A@B = -960*cnt (the usual slice starts at col 128, the
                # clamped-trim slice at 0)
                bbig = const_pool.tile([64, 2, 640], f8)
                nc.sync.dma_start(bbig, BB_d[:])
                WO_t = const_pool.tile([128, NP, D_], f32r)
                nc.sync.dma_start(
                    WO_t, WO_d.rearrange("(c p) n -> p c n", p=128).bitcast(f32r))
                nc.sync.dma_start(
                    V_t.rearrange("p kc (h e) -> p (kc h) e", e=65)[:, :, 64:65],
                    ON_d[:, :, None].bitcast(f32r))

            if debug_taps:
                nc.sync.dma_start(VD[:], V_t.bitcast(f32))

            # ---- phase 2: attention + output projection, per q-tile ----
            with tc.tile_pool(name="att", bufs=6) as att_pool, \
                 tc.tile_pool(name="nrm", bufs=2) as nrm_pool, \
                 tc.tile_pool(name="ps", bufs=4, space="PSUM") as ps_pool:
                def emit_oproj(qt, ctxn, sss=(0, 1, 2, 3)):
                    NOUT = max(1, D_ // 512)
                    OW = min(512, D_)
                    for ss in sss:
                        yp = ps_pool.tile([128, 1024], f32, tag="ps")
                        for c in range(NP):
                            for n in range(NOUT):
                                nc.tensor.matmul(
                                    yp[:, n * OW:(n + 1) * OW],
                                    lhsT=ctxn[c][:, ss * 128:(ss + 1) * 128],
                                    rhs=WO_t[:, c, n * OW:(n + 1) * OW],
                                    start=(c == 0), stop=(c == NP - 1))
                        ys = nrm_pool.tile([128, NOUT * OW], f32, tag="ys")
                        if ss % 2 == 0:
                            nc.scalar.copy(out=ys, in_=yp[:, 0:NOUT * OW])
                        else:
                            nc.vector.tensor_copy(out=ys, in_=yp[:, 0:NOUT * OW])
                        nc.sync.dma_start(
                            Y_d[qt * 512 + ss * 128: qt * 512 + (ss + 1) * 128, :],
                            ys)

                prev = None
                qt_order = [2, 3, 1, 0] if NQT == 4 else list(range(NQT))
                for qt in qt_order:
                    nkc = 4 * qt + 4
                    ctxn = [nrm_pool.tile([128, 512], f32r, tag=f"ctxn{c}",
                                          name=f"ctxn{c}_{qt}")
                            for c in range(NP)]
                    for c in range(NP):
                        ctx = ps_pool.tile([128, 1024], f32, tag="ps")
                        for kc in range(nkc):
                            jp = kc - 4 * qt
                            # f32r matmuls run at 1/4 rate below N=256, so
                            # never trim past 256; the mask matmul zeroes the
                            # extra (fully-masked) columns exactly via exp.
                            trim = min(128 * jp, 256) if jp >= 0 else 0
                            shift = (128 * jp - trim) if jp >= 0 else 0
                            qs = qt * 512
                            stp = ps_pool.tile([128, 1024], f32, tag="ps")
                            st2 = stp.rearrange("p (i n) -> p i n", n=512)
                            for i in (0, 1):
                                nc.tensor.matmul(
                                    st2[:, i, trim:512],
                                    lhsT=KDR[i][32 * c:32 * c + 32, :,
                                                kc * 128:(kc + 1) * 128],
                                    rhs=QDR[i][32 * c:32 * c + 32, :,
                                               qs + trim:qs + 512],
                                    start=True, stop=(jp < 0),
                                    perf_mode=DR, skip_group_check=True,
                                    tile_position=(32 * c, 0))
                            if jp >= 0:
                                # shift>0 (clamped trim): use A2 = (r <= k)
                                # with B shifted one col so the k=0 row of
                                # the fully-masked band still masks.
                                mA = atri2 if shift else atri
                                blo = 1 if shift else 128
                                for i in (0, 1):
                                    nc.tensor.matmul(
                                        st2[:, i, trim:512],
                                        lhsT=mA,
                                        rhs=bbig[:, :, blo:blo + 512 - trim],
                                        start=False, stop=True,
                                        perf_mode=DR, skip_group_check=True)
                            pt = att_pool.tile([128, 2, 512], f32r, tag="pt")
                            nc.scalar.activation(
                                pt[:, :, trim:512], st2[:, :, trim:512],
                                Exp, scale=SM_SCALE)
                            if debug_taps and qt == 0 and c == 0 and kc == 0:
                                nc.sync.dma_start(PTD[:], pt.bitcast(f32))
                            for i in (0, 1):
                                h = 2 * c + i
                                nc.tensor.matmul(
                                    ctx[0:65, i * 512 + trim:i * 512 + 512],
                                    lhsT=V_t[:, kc, h * 65:(h + 1) * 65],
                                    rhs=pt[:, i, trim:512],
                                    start=(kc == 0), stop=(kc == nkc - 1),
                                    skip_group_check=True)
                        if debug_taps and qt == 0 and c == 0:
                            ctxdump = nrm_pool.tile([128, 1024], f32,
                                                    tag="ctxdump")
                            nc.vector.tensor_copy(out=ctxdump, in_=ctx)
                            nc.sync.dma_start(CTXD[:], ctxdump)
                        # denominators live in psum row 64; stage to SBUF on
                        # the same partition, DMA-shift to partition 0 (the
                        # custom-DVE recip and gpsimd broadcast only handle
                        # base-partition-0 inputs on HW), then broadcast.
                        den = nrm_pool.tile([65, 1024], f32, tag="den")
                        nc.vector.tensor_copy(out=den[64:65, :],
                                              in_=ctx[64:65, 0:1024])
                        den0 = nrm_pool.tile([1, 1024], f32, tag="den0")
                        nc.sync.dma_start(den0, den[64:65, :])
                        recip = nrm_pool.tile([1, 1024], f32, tag="recip")
                        nc.vector.reciprocal_approx_fast(out=recip, in_=den0)
                        bca = nrm_pool.tile([64, 512], f32, tag="bca")
                        bcb = nrm_pool.tile([64, 512], f32, tag="bcb")
                        nc.gpsimd.partition_broadcast(bca, recip[0:1, 0:512])
                        nc.gpsimd.partition_broadcast(bcb, recip[0:1, 512:1024])
                        if debug_taps and qt == 0 and c == 0:
                            nc.sync.dma_start(DEND[:], den[64:65, :])
                            nc.sync.dma_start(RECD[:], recip)
                            nc.sync.dma_start(BCAD[:], bca)
                        nc.vector.tensor_mul(
                            ctxn[c][0:64, :], ctx[0:64, 0:512], bca)
                        tmpb = nrm_pool.tile([64, 512], f32r, tag="tmpb")
                        nc.vector.tensor_mul(
                            tmpb, ctx[0:64, 512:1024], bcb)
                        nc.sync.dma_start(ctxn[c][64:128, :], tmpb)
                        if c == 1 and prev is not None:
                            emit_oproj(*prev)
                            prev = None

                    if debug_taps and qt == 0:
                        for c in range(NP):
                            nc.sync.dma_start(CTXND[:, c, :],
                                              ctxn[c].bitcast(f32))
                    prev = (qt, ctxn)
                if prev is not None:
                    emit_oproj(*prev)

    nc.compile()
    return nc


_NC_CACHE = {}


def _get_nc():
    key = (S, D)
    if key not in _NC_CACHE:
        _NC_CACHE[key] = build_mha_kernel(S, D)
    return _NC_CACHE[key]


def make_consts(S_):
    import ml_dtypes

    f8 = ml_dtypes.float8_e4m3

    def fold(m):
        # [128, w] -> [64, 2, w] with row r = 64i+p at (p, i)
        return np.ascontiguousarray(
            m.reshape(2, 64, -1).transpose(1, 0, 2)).astype(f8)

    r = np.arange(128)
    at = fold(4.0 * (r[:, None] < r[None, :]))                 # A[r,k] = r < k
    at2 = fold(4.0 * (r[:, None] <= r[None, :]))               # A2[r,k] = r <= k
    bb = fold(np.where(r[:, None] >= np.arange(640)[None, :] - 128,
                       np.float32(-MBIG), np.float32(0.0)))
    on = np.ones((128, S_ // 128 * GH), dtype=np.float32)
    return at, at2, bb, on


def shard_inputs(X, Wq, Wk, Wv, Wo):
    """Build the 8 per-core input maps from full inputs."""
    X = np.asarray(X, dtype=np.float32)
    Wq = np.asarray(Wq, dtype=np.float32)
    Wk = np.asarray(Wk, dtype=np.float32)
    Wv = np.asarray(Wv, dtype=np.float32)
    Wo = np.asarray(Wo, dtype=np.float32)
    at, at2, bb, on = make_consts(S)
    in_maps = []
    for c in range(NCORES):
        b, g = c // 2, c % 2
        in_maps.append({
            "XT": np.ascontiguousarray(X[b].T),
            "WQ": np.ascontiguousarray(Wq[:, g * GW:(g + 1) * GW]),
            "WK": np.ascontiguousarray(Wk[:, g * GW:(g + 1) * GW]),
            "WV": np.ascontiguousarray(Wv[:, g * GW:(g + 1) * GW]),
            "WO": np.ascontiguousarray(Wo[g * GW:(g + 1) * GW, :]),
            "AT": at, "A2": at2, "BB": bb, "ON": on,
        })
    return in_maps


def kernel(X, Wq, Wk, Wv, Wo, bo):
    from concourse.bass_utils import run_bass_kernel_spmd

    nc = _get_nc()
    in_maps = shard_inputs(X, Wq, Wk, Wv, Wo)
    res = run_bass_kernel_spmd(nc, in_maps, core_ids=list(range(NCORES)))
    bo = np.asarray(bo, dtype=np.float32)
    Y = np.empty((B, S, D), dtype=np.float32)
    for b in range(B):
        Y[b] = res.results[2 * b]["Y"] + res.results[2 * b + 1]["Y"] + bo
    return Y

